# revision 24
# baseline (speedup 1.0000x reference)
"""Single-head attention (B=4, S=4096, F=H=1024) on 8 TRN2 NeuronCores.

Sharding: core = 2*b + h owns batch b, sequence-half h (rows h*2048 ..
(h+1)*2048). Each core projects K/Q/V only for its OWN 2048 rows (bf16
matmuls, fp32 PSUM), then the two cores of a batch exchange K^T and V with
pair-wise AllGathers (2-core replica groups), slab-granular so comm hides
behind compute.

Attention runs in fp8 e4m3 with DoubleRow perf mode (2 fp8 k-planes per
matmul). Accuracy is kept inside the rel-err budget with mean extraction:
  E = exp(s/32) = 1 + P~,   out = (1 x colsum(V) + P~ @ V) / (4096 + sum P~)
P~ = E-1 (|P~|~0.35 vs |E|~1.1) and V are quantized to e4m3 (x16 scale);
the rank-1 mean term uses an exact f32 colsum(V) precomputed on host
(vsum256 input), so fp8 error only rides on the small deviation part.
Q^T/K^T are stored as e4m3 (x16) directly from the projection PSUM.

Phase B processes all 2048 queries per key-chunk pass: each stationary
kts load feeds 4 N=512 matmuls, exps (16 tiles [128,2,2048] e4m3) stay
resident in SBUF, K^T DMA runs once. exp is computed as etmp=exp(s+ln16)
= 16E on the scalar engine, then e8 = etmp-16 (= 16*P~) on the DVE.
"""

import math

import numpy as np
import ml_dtypes

# bass_utils' trace path imports antenv.axon_hooks, which some images lack;
# provide a no-op fallback so an externally-set BASS_TRACE cannot crash us.
try:
    import antenv.axon_hooks  # noqa: F401
except Exception:  # pragma: no cover
    try:
        import sys as _sys
        import types as _types

        import antenv as _antenv

        _m = _types.ModuleType("antenv.axon_hooks")
        _m.set_axon_ntff_profile_hook = lambda h: None
        _m.get_axon_ntff_profile_hook = lambda: None
        _sys.modules["antenv.axon_hooks"] = _m
        _antenv.axon_hooks = _m
    except Exception:
        pass

import concourse.bass as bass  # noqa: F401  (registers engine types)
import concourse.mybir as mybir
import concourse.tile as tile
from concourse import bacc
from concourse.bass_utils import run_bass_kernel_spmd

BF16 = mybir.dt.bfloat16
F32 = mybir.dt.float32
E4 = mybir.dt.float8e4
AF = mybir.ActivationFunctionType
DR = mybir.MatmulPerfMode.DoubleRow

B, S, F, H = 4, 4096, 1024, 1024
QH = S // 2  # rows owned per core
FC = F // 128  # 8 feature chunks
HC = H // 128  # 8 hidden chunks
N_CORES = 8
PAIRS = [[0, 1], [2, 3], [4, 5], [6, 7]]

QSC = 16.0  # q/k/v/P~ fp8 storage scale
EXP_SCALE = 1.0 / (32.0 * QSC * QSC)  # PSUM scores are 256*(q.k); softmax /32
LN16 = math.log(QSC)
# AV psum = (16P~)@(16V) = 256*(P~@V); den_scaled = 256*den = 16*osum + 256*S
DEN_MUL = QSC
DEN_ADD = float(S) * QSC * QSC

# superchunks (256 keys each): (slab, half, sc2); slab-0 first for gather slack
SUPS = [
    (slab, half, sc2) for slab in range(2) for half in range(2) for sc2 in range(4)
]

_NC_CACHE = None


def _build_nc():
    nc = bacc.Bacc("TRN2", target_bir_lowering=False, debug=False)

    xt_ext = nc.declare_dram_parameter("xt", [F, QH], BF16, isOutput=False)
    wq_ext = nc.declare_dram_parameter("wq", [F, H], BF16, isOutput=False)
    wk_ext = nc.declare_dram_parameter("wk", [F, H], BF16, isOutput=False)
    wv_ext = nc.declare_dram_parameter("wv", [F, H], BF16, isOutput=False)
    bqt_ext = nc.declare_dram_parameter("bqt16", [128, HC], F32, isOutput=False)
    bkt_ext = nc.declare_dram_parameter("bkt16", [128, HC], F32, isOutput=False)
    bvr_ext = nc.declare_dram_parameter("bv16rep", [128, H], BF16, isOutput=False)
    vs_ext = nc.declare_dram_parameter("vsum256rep", [128, H], F32, isOutput=False)
    out_ext = nc.declare_dram_parameter("out", [QH, H], BF16, isOutput=True)

    xt_v = xt_ext[:].rearrange("(c p) s -> p c s", p=128)
    wq_v = wq_ext[:].rearrange("(c p) h -> p c h", p=128)
    wk_v = wk_ext[:].rearrange("(c p) h -> p c h", p=128)
    wv_v = wv_ext[:].rearrange("(c p) h -> p c h", p=128)

    with tile.TileContext(nc) as tc:
        with (
            tc.tile_pool(name="const", bufs=1) as constp,
            tc.tile_pool(name="qtres", bufs=1) as qtpool,
            tc.tile_pool(name="ktsp", bufs=4) as ktsp,
            tc.tile_pool(name="spill", bufs=1, space="DRAM") as dramp,
        ):
            ones8_col = constp.tile([128, 2, 1], E4, tag="ones8", name="ones8_col")
            nc.vector.memset(ones8_col[:], 1.0)
            ln16_b = constp.tile([128, 1], F32, tag="ln16", name="ln16_b")
            nc.vector.memset(ln16_b[:], LN16)
            bqt = constp.tile([128, HC], F32, tag="bqt", name="bqt")
            bkt = constp.tile([128, HC], F32, tag="bkt", name="bkt")
            bvr_sb = constp.tile([128, H], BF16, tag="bvr", name="bvr_sb")

            # per-slab own spills + gathered pair buffers (plain Local DRAM)
            kt_own = [
                dramp.tile([HC, 128, 1024], E4, tag=f"kto{s}", name=f"kt_own{s}")
                for s in range(2)
            ]
            v_own = [
                dramp.tile([1024, H], E4, tag=f"vo{s}", name=f"v_own{s}")
                for s in range(2)
            ]
            kt_gath = [
                dramp.tile([2, HC, 128, 1024], E4, tag=f"ktg{s}", name=f"kt_gath{s}")
                for s in range(2)
            ]
            v_gath = [
                dramp.tile([2, 1024, H], E4, tag=f"vg{s}", name=f"v_gath{s}")
                for s in range(2)
            ]

            qt_res = qtpool.tile([128, HC, QH], E4, tag="qtres", name="qt_res")

            def pair_gather(dst, src):
                nc.gpsimd.collective_compute(
                    "AllGather", mybir.AluOpType.bypass, replica_groups=PAIRS,
                    ins=[src.opt()], outs=[dst.opt()],
                )

            # ---------- Phase A: own-half projections in one x^T pass ----------
            with (
                tc.tile_pool(name="wp", bufs=1) as wp,
                tc.tile_pool(name="xp", bufs=2) as xp,
                tc.tile_pool(name="stage", bufs=2) as stp,
                tc.tile_pool(name="psA", bufs=6, space="PSUM") as psA,
            ):
                wk_sb = wp.tile([128, FC, H], BF16, tag="wk", name="wk_sb")
                wq_sb = wp.tile([128, FC, H], BF16, tag="wq", name="wq_sb")
                wv_sb = wp.tile([128, FC, H], BF16, tag="wv", name="wv_sb")
                # first K psum (hh=0) needs only wk cols 0:128 + xts0 cols
                # 0:512 -- land those first, in fine pieces, so the K matmul
                # stream starts early and stays fed while the rest arrives
                nc.sync.dma_start(wk_sb[:, :, 0:128], wk_v[:, :, 0:128])

                xts_l = []
                for sp in range(QH // 1024):  # 1024-column slabs of own x^T
                    xts = xp.tile([128, FC, 1024], BF16, tag="xts", name=f"xts{sp}")
                    xts_l.append(xts)
                    base = sp * 1024
                    if sp == 0:
                        nc.sync.dma_start(xts[:, :, 0:512], xt_v[:, :, 0:512])
                        nc.sync.dma_start(bkt[:], bkt_ext[:])
                        for hh in range(1, HC):  # remaining wk in 128-col pieces
                            nc.sync.dma_start(
                                wk_sb[:, :, hh * 128 : (hh + 1) * 128],
                                wk_v[:, :, hh * 128 : (hh + 1) * 128],
                            )
                        nc.sync.dma_start(xts[:, :, 512:1024], xt_v[:, :, 512:1024])
                        nc.sync.dma_start(bqt[:], bqt_ext[:])
                        nc.sync.dma_start(bvr_sb[:], bvr_ext[:])
                        nc.sync.dma_start(wv_sb[:], wv_v)
                        nc.sync.dma_start(wq_sb[:], wq_v)
                    else:
                        nc.sync.dma_start(
                            xts[:, :, 0:512], xt_v[:, :, base : base + 512]
                        )
                        nc.sync.dma_start(
                            xts[:, :, 512:1024], xt_v[:, :, base + 512 : base + 1024]
                        )

                # K^T both slabs first, so both pair-gathers start early
                for sp in range(QH // 1024):
                    xts = xts_l[sp]
                    kst0 = stp.tile([128, HC, 512], E4, tag="kst", name=f"ksa{sp}")
                    kst1 = stp.tile([128, HC, 512], E4, tag="kst", name=f"ksb{sp}")
                    for hh in range(HC):
                        ps0 = psA.tile([128, 512], F32, tag="psA", name=f"pk0_{sp}_{hh}")
                        ps1 = psA.tile([128, 512], F32, tag="psA", name=f"pk1_{sp}_{hh}")
                        for f in range(FC):
                            lhs = wk_sb[:, f, hh * 128 : (hh + 1) * 128]
                            nc.tensor.matmul(
                                ps0[:], lhs, xts[:, f, 0:512],
                                start=(f == 0), stop=(f == FC - 1),
                            )
                            nc.tensor.matmul(
                                ps1[:], lhs, xts[:, f, 512:1024],
                                start=(f == 0), stop=(f == FC - 1),
                            )
                        bias = bkt[:, hh : hh + 1]
                        nc.scalar.activation(
                            kst0[:, hh, :], ps0[:], AF.Identity, bias=bias, scale=QSC
                        )
                        nc.scalar.activation(
                            kst1[:, hh, :], ps1[:], AF.Identity, bias=bias, scale=QSC
                        )
                    nc.sync.dma_start(
                        kt_own[sp][:, :, 0:512].rearrange("c p q -> p c q"), kst0[:]
                    )
                    nc.sync.dma_start(
                        kt_own[sp][:, :, 512:1024].rearrange("c p q -> p c q"), kst1[:]
                    )
                    pair_gather(kt_gath[sp], kt_own[sp])

                # V both slabs
                for sp in range(QH // 1024):
                    xts = xts_l[sp]
                    vst = stp.tile([128, 8, H], E4, tag="vst", bufs=2, name=f"vst{sp}")
                    for sc in range(8):
                        ps0 = psA.tile([128, 512], F32, tag="psA", name=f"pv0_{sp}_{sc}")
                        ps1 = psA.tile([128, 512], F32, tag="psA", name=f"pv1_{sp}_{sc}")
                        for f in range(FC):
                            lhs = xts[:, f, sc * 128 : (sc + 1) * 128]
                            nc.tensor.matmul(
                                ps0[:], lhs, wv_sb[:, f, 0:512],
                                start=(f == 0), stop=(f == FC - 1),
                            )
                            nc.tensor.matmul(
                                ps1[:], lhs, wv_sb[:, f, 512:1024],
                                start=(f == 0), stop=(f == FC - 1),
                            )
                        # vst = 16*(x@Wv) + 16*bv  (bias via DVE, no K=1 matmul)
                        nc.vector.scalar_tensor_tensor(
                            vst[:, sc, 0:512], ps0[:], QSC, bvr_sb[:, 0:512],
                            mybir.AluOpType.mult, mybir.AluOpType.add,
                        )
                        nc.vector.scalar_tensor_tensor(
                            vst[:, sc, 512:1024], ps1[:], QSC, bvr_sb[:, 512:1024],
                            mybir.AluOpType.mult, mybir.AluOpType.add,
                        )
                    nc.sync.dma_start(
                        v_own[sp][:].rearrange("(c p) h -> p c h", p=128), vst[:]
                    )
                    pair_gather(v_gath[sp], v_own[sp])

                # prefetch first 4 scores K^T tiles during the Q^T phase
                kts_tiles = {}
                for si in range(4):
                    slab, half, sc2 = SUPS[si]
                    kts = ktsp.tile([128, HC, 256], E4, tag="kts", name=f"kts{si}")
                    nc.sync.dma_start(
                        kts[:],
                        kt_gath[slab][half, :, :, sc2 * 256 : (sc2 + 1) * 256]
                        .rearrange("c p k -> p c k"),
                    )
                    kts_tiles[si] = kts

                # Q^T both slabs -> resident SBUF (e4m3, x16)
                for sp in range(QH // 1024):
                    xts = xts_l[sp]
                    base = sp * 1024
                    for hh in range(HC):
                        ps0 = psA.tile([128, 512], F32, tag="psA", name=f"pq0_{sp}_{hh}")
                        ps1 = psA.tile([128, 512], F32, tag="psA", name=f"pq1_{sp}_{hh}")
                        for f in range(FC):
                            lhs = wq_sb[:, f, hh * 128 : (hh + 1) * 128]
                            nc.tensor.matmul(
                                ps0[:], lhs, xts[:, f, 0:512],
                                start=(f == 0), stop=(f == FC - 1),
                            )
                            nc.tensor.matmul(
                                ps1[:], lhs, xts[:, f, 512:1024],
                                start=(f == 0), stop=(f == FC - 1),
                            )
                        bias = bqt[:, hh : hh + 1]
                        nc.scalar.activation(
                            qt_res[:, hh, base : base + 512], ps0[:],
                            AF.Identity, bias=bias, scale=QSC,
                        )
                        nc.scalar.activation(
                            qt_res[:, hh, base + 512 : base + 1024], ps1[:],
                            AF.Identity, bias=bias, scale=QSC,
                        )

            # ---------- Phase B: fp8 attention over all 2048 queries ----------
            with (
                tc.tile_pool(name="vres", bufs=1) as vpool,
                tc.tile_pool(name="expp", bufs=1) as expp,
                tc.tile_pool(name="etp", bufs=4) as etp,
                tc.tile_pool(name="obp", bufs=3) as obp,
            ):
                # vbig[g]: g = slab*2 + half; DMAs issued after scores start
                vbig = [
                    vpool.tile([128, 8, H], E4, tag=f"vb{g}", name=f"vbig{g}")
                    for g in range(4)
                ]
                vsumrep = vpool.tile([128, H], F32, tag="vsr", name="vsumrep")

                # scores + exp: e8[si] holds 16*P~ for superchunk si
                e8 = [
                    expp.tile([128, 2, QH], E4, tag=f"e{si}", name=f"e8_{si}")
                    for si in range(len(SUPS))
                ]
                with tc.tile_pool(name="psS", bufs=2, space="PSUM") as psS:
                    for si, (slab, half, sc2) in enumerate(SUPS):
                        if si in kts_tiles:
                            kts = kts_tiles[si]
                        else:
                            kts = ktsp.tile(
                                [128, HC, 256], E4, tag="kts", name=f"kts{si}"
                            )
                            nc.sync.dma_start(
                                kts[:],
                                kt_gath[slab][half, :, :, sc2 * 256 : (sc2 + 1) * 256]
                                .rearrange("c p k -> p c k"),
                            )
                        if si == 3:  # V needed only for AV; load behind scores
                            for g in range(4):
                                gs, gh = g // 2, g % 2
                                nc.sync.dma_start(
                                    vbig[g][:],
                                    v_gath[gs][gh].rearrange("(c p) h -> p c h", p=128),
                                )
                            nc.sync.dma_start(vsumrep[:], vs_ext[:])
                        for kk in range(2):
                            pss = [
                                psS.tile(
                                    [128, 512], F32, tag=f"psS{qb}",
                                    name=f"pS_{si}_{kk}_{qb}",
                                )
                                for qb in range(4)
                            ]
                            for c2 in range(4):
                                lhsT = kts[:, 2 * c2 : 2 * c2 + 2,
                                           kk * 128 : (kk + 1) * 128]
                                for qb in range(4):
                                    nc.tensor.matmul(
                                        pss[qb][:], lhsT,
                                        qt_res[:, 2 * c2 : 2 * c2 + 2,
                                               qb * 512 : (qb + 1) * 512],
                                        start=(c2 == 0), stop=(c2 == 3),
                                        perf_mode=DR,
                                    )
                            for qb in range(4):
                                etmp = etp.tile(
                                    [128, 512], F32, tag="et", name=f"et{si}_{kk}_{qb}"
                                )
                                nc.scalar.activation(
                                    etmp[:], pss[qb][:], AF.Exp,
                                    bias=ln16_b[:], scale=EXP_SCALE,
                                )
                                nc.vector.tensor_scalar_add(
                                    e8[si][:, kk, qb * 512 : (qb + 1) * 512],
                                    etmp[:], -QSC,
                                )

                # AV: out = (psum + vsumrep) / den_scaled
                with tc.tile_pool(name="psO", bufs=2, space="PSUM") as psO:
                    for q1 in range(QH // 128):
                        qo = q1 * 128
                        o0 = psO.tile([128, 512], F32, tag="o0", name=f"o0_{q1}")
                        o1 = psO.tile([128, 512], F32, tag="o1", name=f"o1_{q1}")
                        osum = psO.tile([128, 1], F32, tag="osum", name=f"os{q1}")
                        n = len(SUPS)
                        for i, (slab, half, sc2) in enumerate(SUPS):
                            g = slab * 2 + half
                            lhsT = e8[i][:, :, qo : qo + 128]
                            first, last = i == 0, i == n - 1
                            nc.tensor.matmul(
                                osum[:], lhsT, ones8_col[:],
                                start=first, stop=last, perf_mode=DR,
                            )
                            nc.tensor.matmul(
                                o0[:], lhsT,
                                vbig[g][:, 2 * sc2 : 2 * sc2 + 2, 0:512],
                                start=first, stop=last, perf_mode=DR,
                            )
                            nc.tensor.matmul(
                                o1[:], lhsT,
                                vbig[g][:, 2 * sc2 : 2 * sc2 + 2, 512:1024],
                                start=first, stop=last, perf_mode=DR,
                            )
                        dens = obp.tile([128, 1], F32, tag="dens", name=f"dn{q1}")
                        nc.vector.tensor_scalar(
                            dens[:], osum[:], DEN_MUL, DEN_ADD,
                            mybir.AluOpType.mult, mybir.AluOpType.add,
                        )
                        recip = obp.tile([128, 1], F32, tag="recip", name=f"rc{q1}")
                        nc.vector.reciprocal(recip[:], dens[:])
                        outsb = obp.tile([128, H], BF16, tag="outsb", name=f"ou{q1}")
                        tmp = obp.tile([128, H], F32, tag="tmpo", name=f"tp{q1}")
                        nc.vector.tensor_tensor(
                            tmp[:, 0:512], o0[:], vsumrep[:, 0:512],
                            mybir.AluOpType.add,
                        )
                        nc.vector.tensor_tensor(
                            tmp[:, 512:1024], o1[:], vsumrep[:, 512:1024],
                            mybir.AluOpType.add,
                        )
                        nc.vector.tensor_scalar_mul(
                            outsb[:, 0:512], tmp[:, 0:512], recip[:]
                        )
                        nc.vector.tensor_scalar_mul(
                            outsb[:, 512:1024], tmp[:, 512:1024], recip[:]
                        )
                        nc.sync.dma_start(out_ext[qo : qo + 128, :], outsb[:])

    nc.compile()
    return nc


def _get_nc():
    global _NC_CACHE
    if _NC_CACHE is None:
        _NC_CACHE = _build_nc()
    return _NC_CACHE


def _make_in_maps(x, Wq, bq, Wk, bk, Wv, bv):
    bf16 = ml_dtypes.bfloat16
    wq_b = np.asarray(Wq, np.float32).astype(bf16)
    wk_b = np.asarray(Wk, np.float32).astype(bf16)
    wv_b = np.asarray(Wv, np.float32).astype(bf16)
    bqt = np.ascontiguousarray(
        (np.asarray(bq, np.float32) * QSC).reshape(HC, 128).T
    )
    bkt = np.ascontiguousarray(
        (np.asarray(bk, np.float32) * QSC).reshape(HC, 128).T
    )
    bv_rep = np.broadcast_to(
        (np.asarray(bv, np.float32) * QSC).astype(bf16).reshape(1, H), (128, H)
    )
    bv_rep = np.ascontiguousarray(bv_rep)
    x = np.asarray(x, np.float32)
    # exact per-batch colsum of V (x256 for the AV psum scale), in f64->f32
    xsum = x.sum(axis=1, dtype=np.float64)  # [B, F]
    vsum = xsum @ np.asarray(Wv, np.float64) + S * np.asarray(bv, np.float64)
    vsum256 = (256.0 * vsum).astype(np.float32)  # [B, H]
    vs_rep = {
        b: np.ascontiguousarray(
            np.broadcast_to(vsum256[b : b + 1], (128, H))
        )
        for b in range(B)
    }
    in_maps = []
    for core in range(N_CORES):
        b, h = core // 2, core % 2
        xt = np.ascontiguousarray(x[b, h * QH : (h + 1) * QH].T).astype(bf16)
        in_maps.append(
            {
                "xt": xt,
                "wq": wq_b,
                "wk": wk_b,
                "wv": wv_b,
                "bqt16": bqt,
                "bkt16": bkt,
                "bv16rep": bv_rep,
                "vsum256rep": vs_rep[b],
            }
        )
    return in_maps


def run_on_hw(inputs, trace=False, tmpdir=None):
    """Returns (full_output, BassKernelResults)."""
    nc = _get_nc()
    in_maps = _make_in_maps(**inputs)
    res = run_bass_kernel_spmd(
        nc, in_maps, core_ids=list(range(N_CORES)), trace=trace, tmpdir=tmpdir
    )
    out = np.empty((B, S, H), np.float32)
    for core in range(N_CORES):
        b, h = core // 2, core % 2
        out[b, h * QH : (h + 1) * QH] = res.results[core]["out"].astype(np.float32)
    return out, res


def kernel(x, Wq, bq, Wk, bk, Wv, bv):
    out, _ = run_on_hw(
        {"x": x, "Wq": Wq, "bq": bq, "Wk": Wk, "bk": bk, "Wv": Wv, "bv": bv}
    )
    return out


# revision 25
# speedup vs baseline: 1.2223x; 1.2223x over previous
"""Single-head attention (B=4, S=4096, F=H=1024) on 8 TRN2 NeuronCores.

Sharding: core = 2*b + h owns batch b, sequence-half h (rows h*2048 ..
(h+1)*2048). Each core projects K/Q/V only for its OWN 2048 rows (bf16
matmuls, fp32 PSUM), then the two cores of a batch exchange K^T and V with
pair-wise AllGathers (2-core replica groups), slab-granular so comm hides
behind compute.

Attention runs in fp8 e4m3 with DoubleRow perf mode (2 fp8 k-planes per
matmul). Accuracy is kept inside the rel-err budget with mean extraction:
  E = exp(s/32) = 1 + P~,   out = (1 x colsum(V) + P~ @ V) / (4096 + sum P~)
P~ = E-1 (|P~|~0.35 vs |E|~1.1) and V are quantized to e4m3 (x16 scale);
the rank-1 mean term uses an exact f32 colsum(V) precomputed on host
(vsum256 input), so fp8 error only rides on the small deviation part.
Q^T/K^T are stored as e4m3 (x16) directly from the projection PSUM.

Phase B processes all 2048 queries per key-chunk pass: each stationary
kts load feeds 4 N=512 matmuls, exps (16 tiles [128,2,2048] e4m3) stay
resident in SBUF, K^T DMA runs once. exp is computed as etmp=exp(s+ln16)
= 16E on the scalar engine, then e8 = etmp-16 (= 16*P~) on the DVE.
"""

import math

import numpy as np
import ml_dtypes

# bass_utils' trace path imports antenv.axon_hooks, which some images lack;
# provide a no-op fallback so an externally-set BASS_TRACE cannot crash us.
try:
    import antenv.axon_hooks  # noqa: F401
except Exception:  # pragma: no cover
    try:
        import sys as _sys
        import types as _types

        import antenv as _antenv

        _m = _types.ModuleType("antenv.axon_hooks")
        _m.set_axon_ntff_profile_hook = lambda h: None
        _m.get_axon_ntff_profile_hook = lambda: None
        _sys.modules["antenv.axon_hooks"] = _m
        _antenv.axon_hooks = _m
    except Exception:
        pass

import concourse.bass as bass  # noqa: F401  (registers engine types)
import concourse.mybir as mybir
import concourse.tile as tile
from concourse import bacc
from concourse.bass_utils import run_bass_kernel_spmd

BF16 = mybir.dt.bfloat16
F32 = mybir.dt.float32
E4 = mybir.dt.float8e4
AF = mybir.ActivationFunctionType
DR = mybir.MatmulPerfMode.DoubleRow

B, S, F, H = 4, 4096, 1024, 1024
QH = S // 2  # rows owned per core
FC = F // 128  # 8 feature chunks
HC = H // 128  # 8 hidden chunks
N_CORES = 8
PAIRS = [[0, 1], [2, 3], [4, 5], [6, 7]]

QSC = 16.0  # q/k/v/P~ fp8 storage scale
EXP_SCALE = 1.0 / (32.0 * QSC * QSC)  # PSUM scores are 256*(q.k); softmax /32
LN16 = math.log(QSC)
# AV psum = (16P~)@(16V) = 256*(P~@V); den_scaled = 256*den = 16*osum + 256*S
DEN_MUL = QSC
DEN_ADD = float(S) * QSC * QSC

# superchunks (256 keys each): (slab, half, sc2); slab-0 first for gather slack
SUPS = [
    (slab, half, sc2) for slab in range(2) for half in range(2) for sc2 in range(4)
]

_NC_CACHE = None


def _build_nc():
    nc = bacc.Bacc("TRN2", target_bir_lowering=False, debug=False)

    xt_ext = nc.declare_dram_parameter("xt", [F, QH], BF16, isOutput=False)
    wq_ext = nc.declare_dram_parameter("wq", [F, H], BF16, isOutput=False)
    wk_ext = nc.declare_dram_parameter("wk", [F, H], BF16, isOutput=False)
    wv_ext = nc.declare_dram_parameter("wv", [F, H], BF16, isOutput=False)
    bqt_ext = nc.declare_dram_parameter("bqt16", [128, HC], F32, isOutput=False)
    bkt_ext = nc.declare_dram_parameter("bkt16", [128, HC], F32, isOutput=False)
    bvr_ext = nc.declare_dram_parameter("bv16rep", [128, H], BF16, isOutput=False)
    vs_ext = nc.declare_dram_parameter("vsum256rep", [128, H], F32, isOutput=False)
    out_ext = nc.declare_dram_parameter("out", [QH, H], BF16, isOutput=True)

    xt_v = xt_ext[:].rearrange("(c p) s -> p c s", p=128)
    wq_v = wq_ext[:].rearrange("(c p) h -> p c h", p=128)
    wk_v = wk_ext[:].rearrange("(c p) h -> p c h", p=128)
    wv_v = wv_ext[:].rearrange("(c p) h -> p c h", p=128)

    with tile.TileContext(nc) as tc:
        with (
            tc.tile_pool(name="const", bufs=1) as constp,
            tc.tile_pool(name="qtres", bufs=1) as qtpool,
            tc.tile_pool(name="ktsp", bufs=4) as ktsp,
            tc.tile_pool(name="spill", bufs=1, space="DRAM") as dramp,
        ):
            ones8_col = constp.tile([128, 2, 1], E4, tag="ones8", name="ones8_col")
            nc.vector.memset(ones8_col[:], 1.0)
            ln16_b = constp.tile([128, 1], F32, tag="ln16", name="ln16_b")
            nc.vector.memset(ln16_b[:], LN16)
            bqt = constp.tile([128, HC], F32, tag="bqt", name="bqt")
            bkt = constp.tile([128, HC], F32, tag="bkt", name="bkt")
            bvr_sb = constp.tile([128, H], BF16, tag="bvr", name="bvr_sb")

            # per-slab own spills + gathered pair buffers (plain Local DRAM)
            kt_own = [
                dramp.tile([HC, 128, 1024], E4, tag=f"kto{s}", name=f"kt_own{s}")
                for s in range(2)
            ]
            v_own = [
                dramp.tile([1024, H], E4, tag=f"vo{s}", name=f"v_own{s}")
                for s in range(2)
            ]
            kt_gath = [
                dramp.tile([2, HC, 128, 1024], E4, tag=f"ktg{s}", name=f"kt_gath{s}")
                for s in range(2)
            ]
            v_gath = [
                dramp.tile([2, 1024, H], E4, tag=f"vg{s}", name=f"v_gath{s}")
                for s in range(2)
            ]

            qt_res = qtpool.tile([128, HC, QH], E4, tag="qtres", name="qt_res")

            def pair_gather(dst, src):
                nc.gpsimd.collective_compute(
                    "AllGather", mybir.AluOpType.bypass, replica_groups=PAIRS,
                    ins=[src.opt()], outs=[dst.opt()],
                )

            # ---------- Phase A: own-half projections in one x^T pass ----------
            with (
                tc.tile_pool(name="wp", bufs=1) as wp,
                tc.tile_pool(name="xp", bufs=2) as xp,
                tc.tile_pool(name="stage", bufs=2) as stp,
                tc.tile_pool(name="psA", bufs=6, space="PSUM") as psA,
            ):
                wk_sb = wp.tile([128, FC, H], BF16, tag="wk", name="wk_sb")
                wq_sb = wp.tile([128, FC, H], BF16, tag="wq", name="wq_sb")
                wv_sb = wp.tile([128, FC, H], BF16, tag="wv", name="wv_sb")
                # first K psum (hh=0) needs only wk cols 0:128 + xts0 cols
                # 0:512 -- land exactly those first so matmuls start early
                nc.sync.dma_start(wk_sb[:, :, 0:128], wk_v[:, :, 0:128])

                xts_l = []
                for sp in range(QH // 1024):  # 1024-column slabs of own x^T
                    xts = xp.tile([128, FC, 1024], BF16, tag="xts", name=f"xts{sp}")
                    xts_l.append(xts)
                    base = sp * 1024
                    nc.sync.dma_start(xts[:, :, 0:512], xt_v[:, :, base : base + 512])
                    if sp == 0:
                        nc.sync.dma_start(wk_sb[:, :, 128:512], wk_v[:, :, 128:512])
                        nc.sync.dma_start(bkt[:], bkt_ext[:])
                    nc.sync.dma_start(
                        xts[:, :, 512:1024], xt_v[:, :, base + 512 : base + 1024]
                    )
                    if sp == 0:  # defer so the first K matmuls start sooner
                        nc.sync.dma_start(wk_sb[:, :, 512:1024], wk_v[:, :, 512:1024])
                        nc.sync.dma_start(bqt[:], bqt_ext[:])
                        nc.sync.dma_start(bvr_sb[:], bvr_ext[:])
                        nc.sync.dma_start(wv_sb[:], wv_v)
                        nc.sync.dma_start(wq_sb[:], wq_v)

                # K^T both slabs first, so both pair-gathers start early
                for sp in range(QH // 1024):
                    xts = xts_l[sp]
                    kst0 = stp.tile([128, HC, 512], E4, tag="kst", name=f"ksa{sp}")
                    kst1 = stp.tile([128, HC, 512], E4, tag="kst", name=f"ksb{sp}")
                    for hh in range(HC):
                        ps0 = psA.tile([128, 512], F32, tag="psA", name=f"pk0_{sp}_{hh}")
                        ps1 = psA.tile([128, 512], F32, tag="psA", name=f"pk1_{sp}_{hh}")
                        for f in range(FC):
                            lhs = wk_sb[:, f, hh * 128 : (hh + 1) * 128]
                            nc.tensor.matmul(
                                ps0[:], lhs, xts[:, f, 0:512],
                                start=(f == 0), stop=(f == FC - 1),
                            )
                            nc.tensor.matmul(
                                ps1[:], lhs, xts[:, f, 512:1024],
                                start=(f == 0), stop=(f == FC - 1),
                            )
                        bias = bkt[:, hh : hh + 1]
                        nc.scalar.activation(
                            kst0[:, hh, :], ps0[:], AF.Identity, bias=bias, scale=QSC
                        )
                        nc.scalar.activation(
                            kst1[:, hh, :], ps1[:], AF.Identity, bias=bias, scale=QSC
                        )
                    nc.sync.dma_start(
                        kt_own[sp][:, :, 0:512].rearrange("c p q -> p c q"), kst0[:]
                    )
                    nc.sync.dma_start(
                        kt_own[sp][:, :, 512:1024].rearrange("c p q -> p c q"), kst1[:]
                    )
                    pair_gather(kt_gath[sp], kt_own[sp])

                # V both slabs
                for sp in range(QH // 1024):
                    xts = xts_l[sp]
                    vst = stp.tile([128, 8, H], E4, tag="vst", bufs=2, name=f"vst{sp}")
                    for sc in range(8):
                        ps0 = psA.tile([128, 512], F32, tag="psA", name=f"pv0_{sp}_{sc}")
                        ps1 = psA.tile([128, 512], F32, tag="psA", name=f"pv1_{sp}_{sc}")
                        for f in range(FC):
                            lhs = xts[:, f, sc * 128 : (sc + 1) * 128]
                            nc.tensor.matmul(
                                ps0[:], lhs, wv_sb[:, f, 0:512],
                                start=(f == 0), stop=(f == FC - 1),
                            )
                            nc.tensor.matmul(
                                ps1[:], lhs, wv_sb[:, f, 512:1024],
                                start=(f == 0), stop=(f == FC - 1),
                            )
                        # vst = 16*(x@Wv) + 16*bv  (bias via DVE, no K=1 matmul)
                        nc.vector.scalar_tensor_tensor(
                            vst[:, sc, 0:512], ps0[:], QSC, bvr_sb[:, 0:512],
                            mybir.AluOpType.mult, mybir.AluOpType.add,
                        )
                        nc.vector.scalar_tensor_tensor(
                            vst[:, sc, 512:1024], ps1[:], QSC, bvr_sb[:, 512:1024],
                            mybir.AluOpType.mult, mybir.AluOpType.add,
                        )
                    nc.sync.dma_start(
                        v_own[sp][:].rearrange("(c p) h -> p c h", p=128), vst[:]
                    )
                    pair_gather(v_gath[sp], v_own[sp])

                # prefetch first 4 scores K^T tiles during the Q^T phase
                kts_tiles = {}
                for si in range(4):
                    slab, half, sc2 = SUPS[si]
                    kts = ktsp.tile([128, HC, 256], E4, tag="kts", name=f"kts{si}")
                    nc.sync.dma_start(
                        kts[:],
                        kt_gath[slab][half, :, :, sc2 * 256 : (sc2 + 1) * 256]
                        .rearrange("c p k -> p c k"),
                    )
                    kts_tiles[si] = kts

                # Q^T both slabs -> resident SBUF (e4m3, x16)
                for sp in range(QH // 1024):
                    xts = xts_l[sp]
                    base = sp * 1024
                    for hh in range(HC):
                        ps0 = psA.tile([128, 512], F32, tag="psA", name=f"pq0_{sp}_{hh}")
                        ps1 = psA.tile([128, 512], F32, tag="psA", name=f"pq1_{sp}_{hh}")
                        for f in range(FC):
                            lhs = wq_sb[:, f, hh * 128 : (hh + 1) * 128]
                            nc.tensor.matmul(
                                ps0[:], lhs, xts[:, f, 0:512],
                                start=(f == 0), stop=(f == FC - 1),
                            )
                            nc.tensor.matmul(
                                ps1[:], lhs, xts[:, f, 512:1024],
                                start=(f == 0), stop=(f == FC - 1),
                            )
                        bias = bqt[:, hh : hh + 1]
                        nc.scalar.activation(
                            qt_res[:, hh, base : base + 512], ps0[:],
                            AF.Identity, bias=bias, scale=QSC,
                        )
                        nc.scalar.activation(
                            qt_res[:, hh, base + 512 : base + 1024], ps1[:],
                            AF.Identity, bias=bias, scale=QSC,
                        )

            # ---------- Phase B: fp8 attention over all 2048 queries ----------
            with (
                tc.tile_pool(name="vres", bufs=1) as vpool,
                tc.tile_pool(name="expp", bufs=1) as expp,
                tc.tile_pool(name="etp", bufs=4) as etp,
                tc.tile_pool(name="obp", bufs=3) as obp,
            ):
                # vbig[g]: g = slab*2 + half; DMAs issued after scores start
                vbig = [
                    vpool.tile([128, 8, H], E4, tag=f"vb{g}", name=f"vbig{g}")
                    for g in range(4)
                ]
                vsumrep = vpool.tile([128, H], F32, tag="vsr", name="vsumrep")

                # scores + exp: e8[si] holds 16*P~ for superchunk si
                e8 = [
                    expp.tile([128, 2, QH], E4, tag=f"e{si}", name=f"e8_{si}")
                    for si in range(len(SUPS))
                ]
                with tc.tile_pool(name="psS", bufs=2, space="PSUM") as psS:
                    for si, (slab, half, sc2) in enumerate(SUPS):
                        if si in kts_tiles:
                            kts = kts_tiles[si]
                        else:
                            kts = ktsp.tile(
                                [128, HC, 256], E4, tag="kts", name=f"kts{si}"
                            )
                            nc.sync.dma_start(
                                kts[:],
                                kt_gath[slab][half, :, :, sc2 * 256 : (sc2 + 1) * 256]
                                .rearrange("c p k -> p c k"),
                            )
                        if si == 3:  # V needed only for AV; load behind scores
                            for g in range(4):
                                gs, gh = g // 2, g % 2
                                nc.sync.dma_start(
                                    vbig[g][:],
                                    v_gath[gs][gh].rearrange("(c p) h -> p c h", p=128),
                                )
                            nc.sync.dma_start(vsumrep[:], vs_ext[:])
                        for kk in range(2):
                            pss = [
                                psS.tile(
                                    [128, 512], F32, tag=f"psS{qb}",
                                    name=f"pS_{si}_{kk}_{qb}",
                                )
                                for qb in range(4)
                            ]
                            for c2 in range(4):
                                lhsT = kts[:, 2 * c2 : 2 * c2 + 2,
                                           kk * 128 : (kk + 1) * 128]
                                for qb in range(4):
                                    nc.tensor.matmul(
                                        pss[qb][:], lhsT,
                                        qt_res[:, 2 * c2 : 2 * c2 + 2,
                                               qb * 512 : (qb + 1) * 512],
                                        start=(c2 == 0), stop=(c2 == 3),
                                        perf_mode=DR,
                                    )
                            for qb in range(4):
                                etmp = etp.tile(
                                    [128, 512], F32, tag="et", name=f"et{si}_{kk}_{qb}"
                                )
                                nc.scalar.activation(
                                    etmp[:], pss[qb][:], AF.Exp,
                                    bias=ln16_b[:], scale=EXP_SCALE,
                                )
                                nc.vector.tensor_scalar_add(
                                    e8[si][:, kk, qb * 512 : (qb + 1) * 512],
                                    etmp[:], -QSC,
                                )

                # AV: out = (psum + vsumrep) / den_scaled
                with tc.tile_pool(name="psO", bufs=2, space="PSUM") as psO:
                    for q1 in range(QH // 128):
                        qo = q1 * 128
                        o0 = psO.tile([128, 512], F32, tag="o0", name=f"o0_{q1}")
                        o1 = psO.tile([128, 512], F32, tag="o1", name=f"o1_{q1}")
                        osum = psO.tile([128, 1], F32, tag="osum", name=f"os{q1}")
                        n = len(SUPS)
                        for i, (slab, half, sc2) in enumerate(SUPS):
                            g = slab * 2 + half
                            lhsT = e8[i][:, :, qo : qo + 128]
                            first, last = i == 0, i == n - 1
                            nc.tensor.matmul(
                                osum[:], lhsT, ones8_col[:],
                                start=first, stop=last, perf_mode=DR,
                            )
                            nc.tensor.matmul(
                                o0[:], lhsT,
                                vbig[g][:, 2 * sc2 : 2 * sc2 + 2, 0:512],
                                start=first, stop=last, perf_mode=DR,
                            )
                            nc.tensor.matmul(
                                o1[:], lhsT,
                                vbig[g][:, 2 * sc2 : 2 * sc2 + 2, 512:1024],
                                start=first, stop=last, perf_mode=DR,
                            )
                        dens = obp.tile([128, 1], F32, tag="dens", name=f"dn{q1}")
                        nc.vector.tensor_scalar(
                            dens[:], osum[:], DEN_MUL, DEN_ADD,
                            mybir.AluOpType.mult, mybir.AluOpType.add,
                        )
                        recip = obp.tile([128, 1], F32, tag="recip", name=f"rc{q1}")
                        nc.vector.reciprocal(recip[:], dens[:])
                        outsb = obp.tile([128, H], BF16, tag="outsb", name=f"ou{q1}")
                        tmp = obp.tile([128, H], F32, tag="tmpo", name=f"tp{q1}")
                        nc.vector.tensor_tensor(
                            tmp[:, 0:512], o0[:], vsumrep[:, 0:512],
                            mybir.AluOpType.add,
                        )
                        nc.vector.tensor_tensor(
                            tmp[:, 512:1024], o1[:], vsumrep[:, 512:1024],
                            mybir.AluOpType.add,
                        )
                        nc.vector.tensor_scalar_mul(
                            outsb[:, 0:512], tmp[:, 0:512], recip[:]
                        )
                        nc.vector.tensor_scalar_mul(
                            outsb[:, 512:1024], tmp[:, 512:1024], recip[:]
                        )
                        nc.sync.dma_start(out_ext[qo : qo + 128, :], outsb[:])

    nc.compile()
    return nc


def _get_nc():
    global _NC_CACHE
    if _NC_CACHE is None:
        _NC_CACHE = _build_nc()
    return _NC_CACHE


def _make_in_maps(x, Wq, bq, Wk, bk, Wv, bv):
    bf16 = ml_dtypes.bfloat16
    wq_b = np.asarray(Wq, np.float32).astype(bf16)
    wk_b = np.asarray(Wk, np.float32).astype(bf16)
    wv_b = np.asarray(Wv, np.float32).astype(bf16)
    bqt = np.ascontiguousarray(
        (np.asarray(bq, np.float32) * QSC).reshape(HC, 128).T
    )
    bkt = np.ascontiguousarray(
        (np.asarray(bk, np.float32) * QSC).reshape(HC, 128).T
    )
    bv_rep = np.broadcast_to(
        (np.asarray(bv, np.float32) * QSC).astype(bf16).reshape(1, H), (128, H)
    )
    bv_rep = np.ascontiguousarray(bv_rep)
    x = np.asarray(x, np.float32)
    # exact per-batch colsum of V (x256 for the AV psum scale), in f64->f32
    xsum = x.sum(axis=1, dtype=np.float64)  # [B, F]
    vsum = xsum @ np.asarray(Wv, np.float64) + S * np.asarray(bv, np.float64)
    vsum256 = (256.0 * vsum).astype(np.float32)  # [B, H]
    vs_rep = {
        b: np.ascontiguousarray(
            np.broadcast_to(vsum256[b : b + 1], (128, H))
        )
        for b in range(B)
    }
    in_maps = []
    for core in range(N_CORES):
        b, h = core // 2, core % 2
        xt = np.ascontiguousarray(x[b, h * QH : (h + 1) * QH].T).astype(bf16)
        in_maps.append(
            {
                "xt": xt,
                "wq": wq_b,
                "wk": wk_b,
                "wv": wv_b,
                "bqt16": bqt,
                "bkt16": bkt,
                "bv16rep": bv_rep,
                "vsum256rep": vs_rep[b],
            }
        )
    return in_maps


def run_on_hw(inputs, trace=False, tmpdir=None):
    """Returns (full_output, BassKernelResults)."""
    nc = _get_nc()
    in_maps = _make_in_maps(**inputs)
    res = run_bass_kernel_spmd(
        nc, in_maps, core_ids=list(range(N_CORES)), trace=trace, tmpdir=tmpdir
    )
    out = np.empty((B, S, H), np.float32)
    for core in range(N_CORES):
        b, h = core // 2, core % 2
        out[b, h * QH : (h + 1) * QH] = res.results[core]["out"].astype(np.float32)
    return out, res


def kernel(x, Wq, bq, Wk, bk, Wv, bv):
    out, _ = run_on_hw(
        {"x": x, "Wq": Wq, "bq": bq, "Wk": Wk, "bk": bk, "Wv": Wv, "bv": bv}
    )
    return out
